# revision 3
# baseline (speedup 1.0000x reference)
"""Memristive fully-connected layer on 8 Trainium2 NeuronCores.

The reference's differential conductance pair collapses algebraically:
g_pos - g_neg = g_eff = k_cond * weights, and the final rescale divides
K_V * k_cond back out, so the module computes exactly y = x @ w + b.

Strategy: data-parallel over the batch. Each core computes a
(1024 x 4096) @ (4096 x 4096) + b GEMM slice. x and w are converted to
bf16 on the host (error ~2e-3, well inside the 2e-2 gate), which halves
HBM traffic and SBUF footprint and enables the PE's fast-weight-load
path, so LDWEIGHTS (~95 ns) hides fully under each 512-column matmul
(~216 ns). The x shard is pre-transposed on host so stationary-operand
tiles are contiguous; the whole xT shard (8.4 MB) stays resident in
SBUF and w streams from HBM exactly once per core. The device computes
pure x @ w (fp32 PSUM, fp32 output); the bias is folded in on the host
after the gather.

Per core: 8 n-blocks of 512 columns. Blocks 0-6 run the contraction in
k-groups (a 1,1,2,4-k-tile startup ramp on block 0 so the first
matmul's data lands quickly, then 4-tile batches), sweeping all 8
output-row tiles per group so PSUM evictions never stall the PE. The
final block runs k-contiguous per output-row tile against a fully
pre-staged w column block, so its evictions and output stores pipeline
during the block instead of trailing it, and the very last eviction is
split across both HWDGE queues. Startup transfers alternate the two
HWDGE rings per k-group so both carry ~equal bytes. The kernel issues
no gpsimd work at all, keeping the slow Pool engine out of the Tile
start/end barriers. A burst of 64-column throwaway matmuls bridges the
PE from kernel start to first data so the HAM clock gate is released
(2.4 GHz) before real work arrives.
"""

import numpy as np

import concourse.bass as bass  # noqa: F401  (registers engine classes)
import concourse.mybir as mybir
from concourse import bacc, tile
from concourse.bass_utils import run_bass_kernel_spmd

dt = mybir.dt

BATCH, N_IN, N_OUT = 8192, 4096, 4096
NCORES = 8
MB = BATCH // NCORES          # 1024 batch rows per core
P = 128
KT = N_IN // P                # 32 contraction tiles
MT = MB // P                  # 8 output-row tiles per core
NBLK = 512                    # matmul free dim (one PSUM bank)
NB = N_OUT // NBLK            # 8 output-column blocks
KB = 4                        # k-tiles per k-block (per w DMA)
NKB = KT // KB                # 8 k-blocks
WARMUP_MM = 48             # N=64 each: fine-grained PE-busy bridge (~2.6 us)

_cache = {}


def _build():
    nc = bacc.Bacc("TRN2", target_bir_lowering=False, debug=False)
    xT = nc.dram_tensor("xT", [N_IN, MB], dt.bfloat16, kind="ExternalInput")
    w = nc.dram_tensor("w", [N_IN, N_OUT], dt.bfloat16, kind="ExternalInput")
    y = nc.dram_tensor("y", [MB, N_OUT], dt.float32, kind="ExternalOutput")

    xT_r = xT.rearrange("(kt p) m -> p kt m", p=P)    # [128, 32, 1024]
    w_r = w.rearrange("(kt p) n -> p kt n", p=P)      # [128, 32, 4096]
    y_r = y.rearrange("(mt p) n -> p mt n", p=P)      # [128, 8, 4096]

    with tile.TileContext(nc) as tc:
        with (
            tc.tile_pool(name="xtp", bufs=1) as xtp,
            tc.tile_pool(name="wp", bufs=8) as wp,
            tc.tile_pool(name="wps", bufs=3) as wps,
            tc.tile_pool(name="w7p", bufs=1) as w7p,
            tc.tile_pool(name="bp", bufs=1) as bp,
            tc.tile_pool(name="op", bufs=3) as op,
            tc.tile_pool(name="ps", bufs=1, space="PSUM") as ps,
        ):
            # w k-block DMA, 4 k-tiles per transfer on the SP queue.
            # Returns the block as a list of per-k-tile [128, 512] views.
            def w_dma(nb, kb):
                wt = wp.tile([P, KB, NBLK], dt.bfloat16, name=f"wt{KB}")
                nc.sync.dma_start(
                    wt[:],
                    w_r[:, kb * KB:(kb + 1) * KB, nb * NBLK:(nb + 1) * NBLK],
                )
                return [wt[:, kk, :] for kk in range(KB)]

            # variable-size w group DMA (startup ramp); sub-4-tile startup
            # tiles live in their own small pool so the main wt tag can
            # keep 8 rotating slots within SBUF budget
            def w_dma_g(nb, ks, eng=None):
                pool = wp if len(ks) == KB else wps
                wt = pool.tile([P, len(ks), NBLK], dt.bfloat16,
                               name=f"wt{len(ks)}")
                (eng or nc.sync).dma_start(
                    wt[:],
                    w_r[:, ks[0]:ks[0] + len(ks), nb * NBLK:(nb + 1) * NBLK],
                )
                return [wt[:, i, :] for i in range(len(ks))]

            xts = xtp.tile([P, KT, MB], dt.bfloat16, name="xts")

            def xt_dma_g(ks, eng=None):
                (eng or nc.scalar).dma_start(
                    xts[:, ks[0]:ks[0] + len(ks), :],
                    xT_r[:, ks[0]:ks[0] + len(ks), :],
                )

            # HAM warmup: throwaway matmuls on a zeroed tile while the
            # first DMAs are in flight, so real matmuls start at 2.4 GHz.
            warm = bp.tile([P, P], dt.bfloat16, name="warm")
            nc.vector.memset(warm[:], 0.0)
            wpsums = [
                ps.tile([P, NBLK], dt.float32, name=f"ps{i}") for i in range(MT)
            ]
            for i in range(WARMUP_MM):
                nc.tensor.matmul(
                    wpsums[i % MT][:, :64], warm[:, :P], warm[:, :64],
                    start=True, stop=True,
                )

            # Startup DMAs in consumption order with a size ramp (1,1,2,4
            # k-tiles) so the first matmul's 0.4 MB of data lands quickly
            # while descriptor generation (~0.8 us per transfer per
            # sequencer) stays off the critical path. Each group's w and xT
            # ride DIFFERENT HWDGE rings, alternating per group, so both
            # rings carry ~equal bytes and each group's two halves move in
            # parallel.
            nb0_groups = [[0], [1], [2, 3], [4, 5, 6, 7]] + [
                list(range(s, s + KB)) for s in range(2 * KB if KB == 4 else KB,
                                                      KT, KB)
            ]
            nb0_wts = []
            for gi, ks in enumerate(nb0_groups):
                weng, xeng = ((nc.sync, nc.scalar) if gi % 2 == 0
                              else (nc.scalar, nc.sync))
                nb0_wts.append(w_dma_g(0, ks, eng=weng))
                if gi == 0:
                    # First xT transfer carries only the m=0 stationary tile
                    # (32 KB) so the first real matmul's data lands ~2 us
                    # sooner; the rest of the k=0 row follows immediately.
                    xeng.dma_start(xts[:, 0:1, 0:P], xT_r[:, 0:1, 0:P])
                    xeng.dma_start(xts[:, 0:1, P:MB], xT_r[:, 0:1, P:MB])
                else:
                    xt_dma_g(ks, eng=xeng)

            # Final n-block's w column: fully staged ahead of time so the
            # last block can run k-contiguous per m-tile. 4 transfers of
            # 8 k-tiles (0.5 MB), emitted interleaved into nb=5's stream.
            w7 = w7p.tile([P, KT, NBLK], dt.bfloat16, name="w7t")

            def w7_dma(i):
                nc.sync.dma_start(
                    w7[:, i * 8:(i + 1) * 8, :],
                    w_r[:, i * 8:(i + 1) * 8, (NB - 1) * NBLK:NB * NBLK],
                )

            # k-group schedule: nb=0 uses the startup ramp groups.
            groups = [list(range(kb * KB, (kb + 1) * KB)) for kb in range(NKB)]

            for nb in range(NB - 1):
                psums = [
                    ps.tile([P, NBLK], dt.float32, name=f"ps{m}")
                    for m in range(MT)
                ]
                ot = None
                gs = nb0_groups if nb == 0 else groups
                for gi, ks in enumerate(gs):
                    if nb == 0:
                        wts = nb0_wts[gi]
                    else:
                        wts = w_dma(nb, ks[0] // KB)
                        if nb == 5 and gi % 2 == 1:
                            w7_dma(gi // 2)
                    last_group = gi == len(gs) - 1
                    for m in range(MT):
                        for kk, k in enumerate(ks):
                            nc.tensor.matmul(
                                psums[m][:],
                                xts[:, k, m * P:(m + 1) * P],
                                wts[kk],
                                start=(k == 0),
                                stop=(k == KT - 1),
                            )
                        if last_group:
                            if m % 2 == 0:
                                ot = op.tile([P, 2, NBLK], dt.float32, name="ot")
                            nc.vector.tensor_copy(
                                ot[:, m % 2, :],
                                psums[m][:],
                            )
                            if m % 2 == 1:
                                nc.scalar.dma_start(
                                    y_r[:, m - 1:m + 1, nb * NBLK:(nb + 1) * NBLK],
                                    ot[:],
                                )

            # Final n-block: k-contiguous per m-tile against resident w7.
            # Each m-tile's 32-matmul accumulation completes ~7 us apart,
            # so the bias-add + store of m pipelines under m+1's matmuls
            # and only the last m's eviction trails the final matmul.
            nb = NB - 1
            for m in range(MT):
                psum = ps.tile([P, NBLK], dt.float32, name=f"ps{m}")
                for k in range(KT):
                    nc.tensor.matmul(
                        psum[:],
                        xts[:, k, m * P:(m + 1) * P],
                        w7[:, k, :],
                        start=(k == 0),
                        stop=(k == KT - 1),
                    )
                ot = op.tile([P, 1, NBLK], dt.float32, name="ot1")
                if m == MT - 1:
                    # last eviction is the kernel tail: one full-width DVE
                    # copy (serial quarter-copies cost more than they save),
                    # then two half stores whose descriptor generation runs
                    # on BOTH sequencers in parallel
                    h = NBLK // 2
                    nc.vector.tensor_copy(ot[:, 0, :], psum[:])
                    for hi, eng in enumerate((nc.sync, nc.scalar)):
                        eng.dma_start(
                            y_r[:, m:m + 1, nb * NBLK + hi * h:nb * NBLK + (hi + 1) * h],
                            ot[:, :, hi * h:(hi + 1) * h],
                        )
                else:
                    nc.vector.tensor_copy(
                        ot[:, 0, :],
                        psum[:],
                    )
                    eng = nc.scalar if m % 2 else nc.sync
                    eng.dma_start(
                        y_r[:, m:m + 1, nb * NBLK:(nb + 1) * NBLK],
                        ot[:],
                    )
    nc.compile()
    return nc


def kernel(x, w, b, _trace=False, _trace_kwargs=None):
    import ml_dtypes

    if "nc" not in _cache:
        _cache["nc"] = _build()
    nc = _cache["nc"]

    bf16 = ml_dtypes.bfloat16
    w2 = np.ascontiguousarray(np.asarray(w, dtype=np.float32).astype(bf16))
    xT_all = np.asarray(x, dtype=np.float32).T.astype(bf16)  # [N_IN, BATCH]
    in_maps = []
    for c in range(NCORES):
        xs = np.ascontiguousarray(xT_all[:, c * MB:(c + 1) * MB])
        in_maps.append({"xT": xs, "w": w2})

    res = run_bass_kernel_spmd(
        nc,
        in_maps,
        core_ids=list(range(NCORES)),
        trace=_trace,
        **(_trace_kwargs or {}),
    )
    out = np.concatenate([res.results[c]["y"] for c in range(NCORES)], axis=0)
    # bias folds in on the host: the device computes x @ w only
    out += np.asarray(b, dtype=np.float32).reshape(1, N_OUT)
    if _trace:
        return out, res
    return out

